# revision 29
# baseline (speedup 1.0000x reference)
"""Trainium2 Bass kernel for 3x3 VALID conv (NCHW, stride 1) via banded-Toeplitz GEMM.

Full input (64, 8, 256, 256) f32 + filter (8, 8, 3, 3) -> output (64, 8, 254, 254).

Sharding: 2-way over batch x 4-way over output rows (core = ns*4 + hs).
Each core handles 32 images x 64 output rows (row shard 3 starts at 190 and
recomputes rows 190-191 so every shard runs the identical program).

PE: SBUF partition (h*8+c) holds ONE input row for the 32 local images.  The
contraction folds ALL THREE filter rows r into the partition dim via a banded
Toeplitz weight W_s[(h,c), (q,m)] = f[m, c, h-q, s], so a block of Q=14
output rows x 2 images needs just 3 matmuls (one per filter column s, N=508).
A block covers 16 image pairs -> 10us of back-to-back PE work, enough to keep
the HAM clock gate at 2.4 GHz.

DMA: with 8 cores running, per-core HBM sustains ~250-380 GB/s, so bytes are
everything: bf16 input (~9 MB) + bf16 output (8.3 MB) per core.  Input
[h, c, n, w] and output [i, m, n, j] layouts (host-side transposes) give
16 KB contiguous descriptors so every SDMA engine runs near its packet-rate
limit.  Loads are issued 2 blocks ahead on the single SWDGE queue so store
semaphore waits never stall load descriptor generation; the first block's
load is split by image group and the last block's store drains per-pair so
the pipeline has almost no serial head or tail.  PE warm-up matmuls on the
weight tile bridge the initial DMA wait so the HAM clock gate reaches
2.4 GHz before the first real matmul; without them the whole kernel runs at
1.2 GHz (throttle release takes ~20 us of sustained PE activity).
Output is cast f32->bf16 on-chip (vector/scalar alternating).
"""

import numpy as np

import concourse.bacc as bacc
import concourse.bass as bass
import concourse.mybir as mybir
import concourse.tile as tile
from concourse import bass_utils

F32 = mybir.dt.float32
BF16 = mybir.dt.bfloat16

N_CORES = 8
N_LOC = 32  # images per core (2-way batch shard)
NROW = 64  # output rows per core (4-way row shard)
C, H, W = 8, 256, 256
M, R, S = 8, 3, 3
HO, WO = H - R + 1, W - S + 1  # 254, 254
Q = 14  # output rows per full block
HB = Q + R - 1  # 16 input rows per full block
QT = 8  # short block outputs (placed LAST so the store tail is small)
HBT = QT + R - 1  # 10
BLOCKS = [(0, Q), (14, Q), (28, Q), (42, Q), (56, QT)]
NB = len(BLOCKS)
NWARM = 14  # dummy PE warm-up matmuls (run while the first load streams in)
PF = 4  # prologue-load ALL blocks so no store packet ever sits ahead of a load in the SWDGE FIFO
HROWS = NROW + R - 1  # 66 input rows per core

_CACHE = {}


def _row_start(hs):
    return 64 * hs if hs < 3 else 190


def _band_weights(f, q_cnt):
    """w[(h,c), s, (q,m)] = f[m, c, h-q, s] for 0 <= h-q < R."""
    hbn = q_cnt + R - 1
    out = np.zeros((hbn * C, S, q_cnt * M), np.float32)
    for c in range(C):
        for m in range(M):
            for q in range(q_cnt):
                for r in range(R):
                    for s in range(S):
                        out[(q + r) * C + c, s, q * M + m] = f[m, c, r, s]
    return out


def _build_program():
    nc = bacc.Bacc("TRN2", target_bir_lowering=False, debug=False)
    x = nc.dram_tensor("x", [HROWS, C, N_LOC, W], BF16, kind="ExternalInput").ap()
    w = nc.dram_tensor("w", [HB * C, S, Q * M], BF16, kind="ExternalInput").ap()
    wt = nc.dram_tensor("wt", [HBT * C, S, QT * M], BF16, kind="ExternalInput").ap()
    y = nc.dram_tensor("y", [NROW, M, N_LOC, WO], BF16, kind="ExternalOutput").ap()

    with tile.TileContext(nc) as tc:
        with (
            tc.tile_pool(name="wpool", bufs=1) as wpool,
            tc.tile_pool(name="xpool", bufs=PF + 1) as xpool,
            tc.tile_pool(name="opool", bufs=3) as opool,
            tc.tile_pool(name="psum", bufs=7, space=bass.MemorySpace.PSUM) as pspool,
            tc.tile_pool(name="warmp", bufs=1, space=bass.MemorySpace.PSUM) as warmpool,
        ):
            wtile = wpool.tile([HB * C, S, Q * M], BF16, tag="w")
            nc.sync.dma_start(wtile[:], w[:])
            wttile = wpool.tile([HBT * C, S, QT * M], BF16, tag="wt")
            nc.sync.dma_start(wttile[:], wt[:])

            # Warm the PE clock gate (HAM) with throwaway matmuls while the
            # first input tiles stream in; source memset by gpsimd, whose
            # queue boots first, so warm-up starts ~2us earlier than the
            # weight-gated variant.
            wsrc = wpool.tile([HB * C, S * Q * M], BF16, tag="wsrc")
            nc.gpsimd.memset(wsrc[:], 1.0)
            warm = warmpool.tile([Q * M, S * Q * M], F32, tag="warm")
            for _ in range(NWARM):
                nc.tensor.matmul(warm[:], wsrc[:, : Q * M], wsrc[:], start=True, stop=True)

            xts = {}

            def issue_load(b):
                i0, q_cnt = BLOCKS[b]
                hbn = q_cnt + R - 1
                xt = xpool.tile([hbn * C, N_LOC, W], BF16, tag="xt")
                if b == 0:
                    # split by image group, finest first, so pair-0 matmuls
                    # can start as soon as the first 4 images have landed
                    for lo, hi in ((0, 4), (4, 8), (8, 16), (16, 32)):
                        nc.gpsimd.dma_start(
                            xt[:, lo:hi, :],
                            x[i0 : i0 + hbn, :, lo:hi, :],
                        )
                else:
                    nc.gpsimd.dma_start(xt[:], x[i0 : i0 + hbn])
                xts[b] = xt

            for b in range(min(PF + 1, NB)):
                issue_load(b)

            for b in range(NB):
                if b + PF + 1 < NB:
                    issue_load(b + PF + 1)
                xt = xts[b]
                i0, q_cnt = BLOCKS[b]
                wsel = wtile if q_cnt == Q else wttile
                mm = q_cnt * M

                last = b == NB - 1
                ot = opool.tile([mm, N_LOC, WO], BF16, tag="ot")
                for p in range(N_LOC // 2):
                    ps = pspool.tile([mm, 2, WO], F32, tag="ps")
                    for s in range(S):
                        nc.tensor.matmul(
                            ps[:],
                            wsel[:, s, :],
                            xt[:, 2 * p : 2 * p + 2, s : s + WO],
                            start=(s == 0),
                            stop=(s == S - 1),
                        )
                    if p % 2 == 0:
                        nc.vector.tensor_copy(ot[:, 2 * p : 2 * p + 2, :], ps[:])
                    else:
                        nc.scalar.copy(ot[:, 2 * p : 2 * p + 2, :], ps[:])
                    if last:
                        # drain the final block's output as the copies land so
                        # almost nothing is left to store after the last MM
                        if p == 7:
                            nc.gpsimd.dma_start(
                                y[i0 : i0 + q_cnt, :, 0:16], ot[:, 0:16, :]
                            )
                        elif p == 11:
                            nc.gpsimd.dma_start(
                                y[i0 : i0 + q_cnt, :, 16:24], ot[:, 16:24, :]
                            )
                        elif p >= 12:
                            j0 = 2 * p
                            nc.gpsimd.dma_start(
                                y[i0 : i0 + q_cnt, :, j0 : j0 + 2],
                                ot[:, j0 : j0 + 2, :],
                            )
                if not last:
                    nc.gpsimd.dma_start(y[i0 : i0 + q_cnt], ot[:])
    nc.compile()
    return nc


def _get_program():
    if "nc" not in _CACHE:
        _CACHE["nc"] = _build_program()
    return _CACHE["nc"]


def _to_bf16(a):
    import ml_dtypes

    return np.ascontiguousarray(np.asarray(a, np.float32)).astype(ml_dtypes.bfloat16)


def _make_inputs(x_full, f):
    """Per-core input dicts. x_full: (64, 8, 256, 256)."""
    w_full = _to_bf16(_band_weights(np.asarray(f, np.float32), Q))
    w_tail = _to_bf16(_band_weights(np.asarray(f, np.float32), QT))
    maps = []
    for core in range(N_CORES):
        ns, hs = divmod(core, 4)
        g0 = _row_start(hs)
        xs = np.asarray(
            x_full[32 * ns : 32 * ns + 32, :, g0 : g0 + HROWS, :], np.float32
        )
        # [n, c, h, w] -> [h, c, n, w]
        xdev = _to_bf16(xs.transpose(2, 1, 0, 3))
        maps.append({"x": xdev, "w": w_full, "wt": w_tail})
    return maps


def _assemble(results):
    out = np.empty((64, 8, HO, WO), np.float32)
    for core, r in enumerate(results):
        ns, hs = divmod(core, 4)
        g0 = _row_start(hs)
        # y: [i, m, n, j] -> [n, m, i, j]
        yt = np.asarray(r["y"]).transpose(2, 1, 0, 3).astype(np.float32)
        lo = 0 if hs < 3 else 2
        out[32 * ns : 32 * ns + 32, :, g0 + lo : g0 + NROW, :] = yt[:, :, lo:, :]
    return out


def kernel(_input, _filter):
    nc = _get_program()
    in_maps = _make_inputs(_input, _filter)
    res = bass_utils.run_bass_kernel_spmd(nc, in_maps, core_ids=list(range(N_CORES)))
    return _assemble(res.results)
